# revision 12
# baseline (speedup 1.0000x reference)
"""Multi-head causal attention (GQA + QK-RMSNorm + RoPE) for Trainium2, 8 cores.

Sharding: 8 cores = 2 batches x 4 head-groups (tensor parallel over heads).
Each core handles one batch and 8 Q heads + 2 KV heads:
  - QKV projection for its head slice (fp16 matmuls, fp32 psum)
  - QK RMSNorm + RoPE (gamma folded into host-side cos/sin tables)
  - causal attention in k-major (transposed-scores) layout:
      sT[k, q] = kT.T @ qT ; p = exp(sT/8) ; ctx = pT.T @ [v | 1]
    (the ones column yields the softmax denominator for free)
  - output projection against its w_out column slice -> partial [S, D]
Host sums the 4 head-group partials per batch.

Layout/engine choices (v2):
  - all transposes (q/k after rope, ctx before out-proj) go through the DMA
    XBAR transpose (SBUF->SBUF, 16x128 tiles) instead of PE transpose +
    psum->SBUF copy; K heads are duplicated in the rope output so the
    transposed kT tile directly carries the [dims x2] layout the paired-head
    score matmuls need
  - attention J-blocks are interleaved into the QKV tile loop (attn_J(J)
    right after token tile 4J+3) so the scalar engine's exp overlaps the
    QKV matmul stream
  - score matmuls on diagonal blocks only stream the causally-live columns
  - cos/sin tables are fp16 so the rope multiplies hit DVE fast modes
"""
import sys
import numpy as np
from contextlib import ExitStack

if '/opt/trn_rl_repo' not in sys.path:
    sys.path.insert(0, '/opt/trn_rl_repo')

import concourse.bacc as bacc
import concourse.tile as tile
import concourse.mybir as mybir
from concourse.bass_utils import run_bass_kernel_spmd

dt = mybir.dt
AF = mybir.ActivationFunctionType
AX = mybir.AxisListType
ALU = mybir.AluOpType

HEAD_DIM = 64
NUM_Q_HEADS = 32
NUM_KV_HEADS = 8
ROPE_FREQ = 10000.0
EPS = 1e-6

B, S, D = 2, 2048, 2048
QH = 8            # q heads per core
KVH = 2           # kv heads per core
N_CORES = 8
ST = S // 128      # 16 token tiles of 128
NJ = S // 512      # 4 big q blocks of 512

F16 = dt.float16
F32 = dt.float32


def _build(repeat=1):
    nc = bacc.Bacc("TRN2", target_bir_lowering=False, debug=False,
                   num_devices=N_CORES)

    xT = nc.dram_tensor("xT", [D, S], F16, kind="ExternalInput").ap()
    wqkvT = nc.dram_tensor("wqkvT", [D, (QH + 2 * KVH) * HEAD_DIM], F16,
                           kind="ExternalInput").ap()          # cols: 512 q | 128 k | 128 v
    woutT = nc.dram_tensor("woutT", [QH * HEAD_DIM, D], F16,
                           kind="ExternalInput").ap()
    cos_q = nc.dram_tensor("cos_q", [S, HEAD_DIM], F16, kind="ExternalInput").ap()
    sin_q = nc.dram_tensor("sin_q", [S, HEAD_DIM], F16, kind="ExternalInput").ap()
    cos_k = nc.dram_tensor("cos_k", [S, HEAD_DIM], F16, kind="ExternalInput").ap()
    sin_k = nc.dram_tensor("sin_k", [S, HEAD_DIM], F16, kind="ExternalInput").ap()
    mask_i = nc.dram_tensor("mask_i", [128, 128], F16, kind="ExternalInput").ap()
    out = nc.dram_tensor("out", [S, D], F32, kind="ExternalOutput").ap()

    with tile.TileContext(nc) as tc, ExitStack() as ctx:
        perm = ctx.enter_context(tc.tile_pool(name="perm", bufs=1))

        # ---- persistent tiles ----
        w_ch = [perm.tile([128, 768], F16, tag=f"w{dc}", name=f"w{dc}") for dc in range(16)]
        wo_ch = [perm.tile([128, D], F16, tag=f"wo{p}", name=f"wo{p}") for p in range(4)]
        tabs = {}
        for nm in ("cq", "sq", "ck", "sk"):
            tabs[nm] = perm.tile([128, ST, HEAD_DIM], F16, tag=f"tab{nm}", name=f"tab{nm}")
        tab_srcs = {"cq": cos_q, "sq": sin_q, "ck": cos_k, "sk": sin_k}
        tmask = perm.tile([128, 128], F16, tag="mask")
        kmagic = perm.tile([128, 10], dt.int32, tag="kmagic")
        nc.vector.memset(kmagic[:], 0x5F3759DF)

        qTa = perm.tile([128, 4, S], F16, tag="qTa", name="qTa")
        kTa = perm.tile([128, KVH, S], F16, tag="kTa", name="kTa")
        vext = [perm.tile([128, KVH, HEAD_DIM + 1], F16, tag=f"v{i}", name=f"v{i}")
                for i in range(ST)]
        for i in range(ST):
            nc.vector.memset(vext[i][:, :, HEAD_DIM:HEAD_DIM + 1], 1.0)
        ctxTa = perm.tile([128, 4, S], F16, tag="ctxTa", name="ctxTa")

        rep_ctx = tc.For_i(0, repeat, 1) if repeat > 1 else None
        if rep_ctx is not None:
            ctx.enter_context(rep_ctx)

        # ---- pools; PSUM budget (8 banks): sT2 2x[128,1024] = 4,
        #      cbank 2x[128,260] = 2, shared po/p1-projection 2x[128,512] = 2
        psum = ctx.enter_context(tc.tile_pool(name="psum", bufs=2, space="PSUM"))
        p1sb = ctx.enter_context(tc.tile_pool(name="p1sb", bufs=4))
        p1st = ctx.enter_context(tc.tile_pool(name="p1st", bufs=2))
        ptpool = ctx.enter_context(tc.tile_pool(name="ptpool", bufs=6))
        cnpool = ctx.enter_context(tc.tile_pool(name="cnpool", bufs=3))
        rcpool = ctx.enter_context(tc.tile_pool(name="rcpool", bufs=8))
        osb = ctx.enter_context(tc.tile_pool(name="osb", bufs=6))

        def load_xt(i2):
            t = p1sb.tile([128, 16, 256], F16, tag="xt", name="xt")
            nc.sync.dma_start(
                t[:], xT[:, 256 * i2:256 * (i2 + 1)]
                .rearrange("(n p) s -> p n s", p=128))
            return t

        xts = {0: load_xt(0)}
        for dc in range(16):
            nc.sync.dma_start(w_ch[dc][:], wqkvT[128 * dc:128 * (dc + 1), :])
        xts[1] = load_xt(1)
        for nm in ("cq", "sq", "ck", "sk"):
            nc.sync.dma_start(tabs[nm][:],
                              tab_srcs[nm].rearrange("(n p) d -> p n d", p=128))
        nc.sync.dma_start(tmask[:], mask_i[:])
        for p in range(4):
            nc.sync.dma_start(wo_ch[p][:], woutT[128 * p:128 * (p + 1), :])

        # ---------------- output projection drain queue ----------------
        po_queue = []

        def emit_po(i, dch):
            po = psum.tile([128, 512], F32, tag="tp", name="po")
            for p in range(4):
                nc.tensor.matmul(
                    po[:], ctxTa[:, p, 128 * i:128 * (i + 1)],
                    wo_ch[p][:, 512 * dch:512 * (dch + 1)],
                    start=(p == 0), stop=(p == 3))
            ob = osb.tile([128, 512], F32, tag="ob", name="ob")
            nc.vector.tensor_copy(ob[:], po[:])
            nc.sync.dma_start(
                out[128 * i:128 * (i + 1), 512 * dch:512 * (dch + 1)],
                ob[:])

        def pop_po(n=1):
            for _ in range(n):
                if po_queue:
                    emit_po(*po_queue.pop(0))

        # ---------------- phase-1 tile as a chunked generator -----------------
        def p1_tile_steps(i):
            """QKV + norm + rope for token tile i, yielding between chunks so
            the caller can interleave attention c-iterations."""
            xt, ii = xts[i // 2], i % 2
            if i % 2 == 0 and (i // 2) + 2 < ST // 2:
                xts[(i // 2) + 2] = load_xt((i // 2) + 2)
            # q projection: [128,512] accumulated in a shared "tp" psum slot
            pq = psum.tile([128, 512], F32, tag="tp", name="pq")
            for dcc in range(4):
                for dc in range(4 * dcc, 4 * dcc + 4):
                    nc.tensor.matmul(pq[:], xt[:, dc, 128 * ii:128 * (ii + 1)],
                                     w_ch[dc][:, 0:512],
                                     start=(dc == 0), stop=(dc == 15))
                yield
            qsb = p1sb.tile([128, 640], F16, tag="qsb")
            nc.scalar.copy(qsb[:, 0:512], pq[:])
            sqt = p1sb.tile([128, 640], F16, tag="sqt")
            nc.scalar.square(sqt[:, 0:512], pq[:])
            yield
            # kv projection: [128,256] in the next shared slot
            pkv = psum.tile([128, 256], F32, tag="tp", name="pkv")
            for dcc in range(2):
                for dc in range(8 * dcc, 8 * dcc + 8):
                    nc.tensor.matmul(pkv[:], xt[:, dc, 128 * ii:128 * (ii + 1)],
                                     w_ch[dc][:, 512:768],
                                     start=(dc == 0), stop=(dc == 15))
                yield
            nc.scalar.copy(qsb[:, 512:640], pkv[:, 0:128])
            nc.scalar.square(sqt[:, 512:640], pkv[:, 0:128])
            nc.scalar.copy(vext[i][:, :, 0:HEAD_DIM],
                           pkv[:, 128:256].rearrange("p (h d) -> p h d", h=2))
            yield
            # --- RMSNorm stats: reduce + Newton rsqrt (DVE)
            ssum = p1st.tile([128, 10], F32, tag="ssum")
            nc.vector.tensor_reduce(
                ssum[:], sqt[:].rearrange("p (h d) -> p h d", h=10),
                axis=AX.X, op=ALU.add)
            # rsqrt(ssum) = 1/(8*rms) -- the x8 in the rope tables completes
            # the normalization. Quake bit-shift seed + 2 Newton iterations
            # is <1e-5 relative for any positive input.
            hh_ = p1st.tile([128, 10], F32, tag="hh")
            nc.vector.tensor_scalar_mul(hh_[:], ssum[:], -0.5)
            sbi = p1st.tile([128, 10], dt.int32, tag="sbi")
            nc.vector.tensor_scalar(sbi[:], ssum[:].bitcast(dt.int32), 1, None,
                                    ALU.arith_shift_right)
            rstd = p1st.tile([128, 10], F32, tag="rstd")
            nc.vector.tensor_sub(rstd[:].bitcast(dt.int32), kmagic[:], sbi[:])
            ita = p1st.tile([128, 10], F32, tag="ita")
            itb = p1st.tile([128, 10], F32, tag="itb")
            for _ in range(2):
                nc.vector.tensor_mul(ita[:], rstd[:], rstd[:])
                nc.vector.tensor_mul(itb[:], ita[:], hh_[:])
                nc.vector.tensor_scalar_add(itb[:], itb[:], 1.5)
                nc.vector.tensor_mul(rstd[:], rstd[:], itb[:])
            yield
            # --- RoPE on unnormalized projections (tables carry x8)
            qru = p1sb.tile([128, 640], F16, tag="qru")
            t2 = p1sb.tile([128, 640], F16, tag="t2")
            qn_q4 = qsb[:, 0:512].rearrange("p (h two x) -> p h two x", h=8, two=2)
            qn_q3 = qsb[:, 0:512].rearrange("p (h d) -> p h d", h=8)
            cosq = tabs["cq"][:, i, :].unsqueeze(1).broadcast_to([128, 8, HEAD_DIM])
            sinq4 = tabs["sq"][:, i, :].unsqueeze(1) \
                .broadcast_to([128, 8, HEAD_DIM]) \
                .rearrange("p h (two x) -> p h two x", two=2)
            t2_q4 = t2[:, 0:512].rearrange("p (h two x) -> p h two x", h=8, two=2)
            nc.vector.tensor_mul(t2_q4[:, :, 0, :], qn_q4[:, :, 1, :], sinq4[:, :, 0, :])
            nc.vector.tensor_mul(t2_q4[:, :, 1, :], qn_q4[:, :, 0, :], sinq4[:, :, 1, :])
            qru_q3 = qru[:, 0:512].rearrange("p (h d) -> p h d", h=8)
            nc.vector.tensor_mul(qru_q3, qn_q3, cosq)
            nc.vector.tensor_add(qru[:, 0:512], qru[:, 0:512], t2[:, 0:512])
            yield
            qn_k4 = qsb[:, 512:640].rearrange("p (h two x) -> p h two x", h=2, two=2)
            qn_k3 = qsb[:, 512:640].rearrange("p (h d) -> p h d", h=2)
            cosk = tabs["ck"][:, i, :].unsqueeze(1).broadcast_to([128, 2, HEAD_DIM])
            sink4 = tabs["sk"][:, i, :].unsqueeze(1) \
                .broadcast_to([128, 2, HEAD_DIM]) \
                .rearrange("p h (two x) -> p h two x", two=2)
            t2k4 = t2[:, 512:640].rearrange("p (h two x) -> p h two x", h=2, two=2)
            qruk = qru[:, 512:640].rearrange("p (h d) -> p h d", h=2)
            nc.vector.tensor_mul(t2k4[:, :, 0, :], qn_k4[:, :, 1, :], sink4[:, :, 0, :])
            nc.vector.tensor_mul(t2k4[:, :, 1, :], qn_k4[:, :, 0, :], sink4[:, :, 1, :])
            nc.vector.tensor_mul(qruk, qn_k3, cosk)
            nc.vector.tensor_add(qru[:, 512:640], qru[:, 512:640], t2[:, 512:640])
            # --- join: qr = qru * rstd; k written into both dup slots
            qr = p1sb.tile([128, 768], F16, tag="qr")
            nc.vector.tensor_mul(
                qr[:, 0:512].rearrange("p (h d) -> p h d", h=8),
                qru[:, 0:512].rearrange("p (h d) -> p h d", h=8),
                rstd[:, 0:8].unsqueeze(2).broadcast_to([128, 8, HEAD_DIM]))
            qrk = qr[:, 512:768].rearrange("p (h dup d) -> p h dup d", h=2, dup=2)
            for d_ in (0, 1):
                nc.vector.tensor_mul(
                    qrk[:, :, d_, :],
                    qru[:, 512:640].rearrange("p (h d) -> p h d", h=2),
                    rstd[:, 8:10].unsqueeze(2).broadcast_to([128, 2, HEAD_DIM]))
            yield
            # --- transposes to dim-major via DMA XBAR (Act hwdge queue)
            nc.sync.dma_start(qTa[:, :, 128 * i:128 * (i + 1)],
                              qr[:, 0:512], transpose=True)
            nc.sync.dma_start(kTa[:, :, 128 * i:128 * (i + 1)],
                              qr[:, 512:768], transpose=True)
            yield

        def p1_group_steps(G):
            for i in range(4 * G, 4 * G + 4):
                yield from p1_tile_steps(i)

        # ---------------- attention J-block, p1 chunks injected per c ----------
        def attn_J(J, p1gen):
            n_c = 4 * (4 * J + 4)          # total c-iters this block
            n_pulled = [0]
            p1_chunks = []
            if p1gen is not None:
                p1_chunks.append(p1gen)

            def pull(frac):
                # keep the injected p1 stream proportional to attn progress
                if not p1_chunks:
                    return
                gen = p1_chunks[0]
                target = int(min(frac / 0.8, 1.0) * 29 * 4)
                while n_pulled[0] < target:
                    try:
                        next(gen)
                        n_pulled[0] += 1
                    except StopIteration:
                        p1_chunks.pop(0)
                        return

            ctxn = [cnpool.tile([128, 512], F16, tag=f"cn{jj}", name=f"cn{jj}")
                    for jj in range(4)]
            ci = [0]
            for hp in range(4):          # heads (2hp, 2hp+1), both use kv g
                g = hp // 2
                cbank = [psum.tile([128, 260], F32, tag="pkv", name=f"cb{w}")
                         for w in (0, 1)]

                def epi(jj):
                    w, loc = jj // 2, jj % 2
                    cb2 = cbank[w][:, 130 * loc:130 * (loc + 1)].rearrange(
                        "p (h e) -> p h e", h=2)
                    rc = rcpool.tile([128, 2], F32, tag="rc", name="rc")
                    nc.vector.reciprocal(rc[:], cb2[:, :, 64:65].squeeze(2))
                    nc.vector.tensor_mul(
                        ctxn[jj][:, 128 * hp:128 * (hp + 1)].rearrange(
                            "p (h d) -> p h d", h=2),
                        cb2[:, :, 0:64],
                        rc[:].unsqueeze(2).broadcast_to([128, 2, HEAD_DIM]))

                def ctx_mms(pt, jj0, c):
                    for jj in range(jj0, 4):
                        w, loc = jj // 2, jj % 2
                        for hh in (0, 1):
                            o = 130 * loc + 65 * hh
                            nc.tensor.matmul(
                                cbank[w][:, o:o + 65],
                                pt[:, 512 * hh + 128 * jj:512 * hh + 128 * (jj + 1)],
                                vext[c][:, g, :],
                                start=(c == 0 and loc == 0 and hh == 0),
                                stop=(c == 4 * J + jj and jj % 2 == 1 and hh == 1),
                                skip_group_check=True)

                pending = None
                for c in range(4 * J + 4):
                    jj0 = max(0, c - 4 * J)
                    sT2 = psum.tile([128, 1024], F32, tag="pq", name="sT2")
                    for hh in (0, 1):
                        nc.tensor.matmul(
                            sT2[:, 512 * hh + 128 * jj0:512 * (hh + 1)],
                            kTa[64 * hh:64 * hh + 64, g, 128 * c:128 * (c + 1)],
                            qTa[64 * hh:64 * hh + 64, hp,
                                512 * J + 128 * jj0:512 * (J + 1)],
                            start=True, stop=True)
                    if pending is not None:
                        ctx_mms(*pending)
                        pending = None
                        if c == 4 * J + 2:   # bank 0 (jj 0,1) is complete
                            epi(0)
                            epi(1)
                    pop_po(1)
                    ci[0] += 1
                    pull(ci[0] / n_c)
                    pt = ptpool.tile([128, 1024], F16, tag="pt", name="pt")
                    ptv = pt[:].rearrange("p (h x) -> p h x", h=2)[:, :, 128 * jj0:512]
                    sTv = sT2[:].rearrange("p (h x) -> p h x", h=2)[:, :, 128 * jj0:512]
                    nc.scalar.activation(ptv, sTv, AF.Exp, scale=0.125)
                    if c >= 4 * J:      # diagonal: triangular mask, both heads
                        dv = pt[:].rearrange("p (h x) -> p h x", h=2)[
                            :, :, 128 * jj0:128 * (jj0 + 1)]
                        nc.vector.tensor_mul(
                            dv, dv,
                            tmask[:].unsqueeze(1).broadcast_to([128, 2, 128]))
                    pending = (pt, jj0, c)
                ctx_mms(*pending)
                pending = None
                for jj in (2, 3):
                    epi(jj)
            # ctx tiles to dim-major via DMA XBAR (batched over p-blocks)
            for jj in range(4):
                nc.sync.dma_start(
                    ctxTa[:, :, 512 * J + 128 * jj:512 * J + 128 * (jj + 1)],
                    ctxn[jj][:, 0:512], transpose=True)
            # drain any p1 remainder, then queue this block's out-projection
            while p1_chunks:
                pull(2.0)
            for i in range(4 * J, 4 * J + 4):
                for dch in range(4):
                    po_queue.append((i, dch))

        # ---------------- main schedule -----------------
        for _ in p1_group_steps(0):
            pass
        for J in range(NJ):
            attn_J(J, p1_group_steps(J + 1) if J + 1 < NJ else None)
        while po_queue:
            emit_po(*po_queue.pop(0))

    nc.compile()
    return nc


_NC = {}


def _get_nc(repeat=1):
    if repeat not in _NC:
        _NC[repeat] = _build(repeat)
    return _NC[repeat]


_RUNNER = {}


def _get_runner(repeat=1):
    """Build (once) a jitted 8-core sharded callable around the bass program.

    Slim replica of bass2jax.run_bass_via_pjrt's multi-core path, kept
    reusable so repeated invocations skip retracing/recompilation.
    """
    if repeat in _RUNNER:
        return _RUNNER[repeat]
    import jax
    from jax.sharding import Mesh, PartitionSpec
    from jax.experimental.shard_map import shard_map
    from concourse import bass2jax
    from concourse import mybir as _mybir

    nc = _get_nc(repeat)
    bass2jax.install_neuronx_cc_hook()

    partition_name = nc.partition_id_tensor.name if nc.partition_id_tensor else None
    in_names, out_names, out_avals, zero_outs = [], [], [], []
    for alloc in nc.m.functions[0].allocations:
        if not isinstance(alloc, _mybir.MemoryLocationSet):
            continue
        name = alloc.memorylocations[0].name
        if alloc.kind == "ExternalInput":
            if name != partition_name:
                in_names.append(name)
        elif alloc.kind == "ExternalOutput":
            shape = tuple(alloc.tensor_shape)
            np_dt = _mybir.dt.np(alloc.dtype)
            out_names.append(name)
            out_avals.append(jax.core.ShapedArray(shape, np_dt))
            zero_outs.append(np.zeros(shape, np_dt))
    n_params = len(in_names)
    all_in_names = list(in_names) + list(out_names)
    if partition_name is not None:
        all_in_names.append(partition_name)

    def _body(*args):
        operands = list(args)
        if partition_name is not None:
            operands.append(bass2jax.partition_id_tensor())
        outs = bass2jax._bass_exec_p.bind(
            *operands,
            out_avals=tuple(out_avals),
            in_names=tuple(all_in_names),
            out_names=tuple(out_names),
            lowering_input_output_aliases=(),
            sim_require_finite=True,
            sim_require_nnan=True,
            nc=nc,
        )
        return tuple(outs)

    devices = jax.devices()[:N_CORES]
    mesh = Mesh(np.asarray(devices), ("core",))
    in_specs = (PartitionSpec("core"),) * (n_params + len(out_names))
    out_specs = (PartitionSpec("core"),) * len(out_names)
    sharded = jax.jit(shard_map(_body, mesh=mesh, in_specs=in_specs,
                                out_specs=out_specs, check_rep=False),
                      keep_unused=True)

    concat_zeros = [np.zeros((N_CORES * z.shape[0], *z.shape[1:]), z.dtype)
                    for z in zero_outs]

    _dev_cache = {}

    def run(in_maps, iters=1, time_list=None, fetch=True):
        import time as _time
        from jax.sharding import NamedSharding
        shard = NamedSharding(mesh, PartitionSpec("core"))
        key = id(in_maps)
        if key not in _dev_cache:
            per_core = [[np.asarray(m[nm]) for nm in in_names] for m in in_maps]
            concat_in = [np.concatenate([per_core[c][i] for c in range(N_CORES)],
                                        axis=0) for i in range(n_params)]
            dev_in = [jax.device_put(a, shard) for a in concat_in]
            dev_zero = [jax.device_put(z, shard) for z in concat_zeros]
            jax.block_until_ready(dev_in)
            _dev_cache.clear()
            _dev_cache[key] = (dev_in, dev_zero)
        dev_in, dev_zero = _dev_cache[key]
        out_arrs = None
        if iters <= 1:
            out_arrs = sharded(*dev_in, *dev_zero)
            jax.block_until_ready(out_arrs)
        else:
            # async batch: submit all, block once; caller computes slope
            sharded(*dev_in, *dev_zero)  # warm
            t0 = _time.perf_counter()
            for _ in range(iters):
                out_arrs = sharded(*dev_in, *dev_zero)
            jax.block_until_ready(out_arrs)
            if time_list is not None:
                time_list.append(_time.perf_counter() - t0)
        if not fetch:
            del out_arrs
            return None
        return [
            {nm: np.asarray(out_arrs[i]).reshape(N_CORES, *out_avals[i].shape)[c]
             for i, nm in enumerate(out_names)}
            for c in range(N_CORES)
        ]

    _RUNNER[repeat] = run
    return run


def _host_tables(q_gamma, k_gamma):
    pos = np.arange(S, dtype=np.float32)
    inv = 1.0 / (ROPE_FREQ ** (np.arange(0, HEAD_DIM, 2, dtype=np.float32)
                               / HEAD_DIM))
    fr = pos[:, None] * inv[None, :]
    emb = np.concatenate([fr, fr], axis=-1)
    cos = np.cos(emb).astype(np.float32)
    sin = np.sin(emb).astype(np.float32)
    outs = []
    for gamma in (q_gamma, k_gamma):
        g = gamma.astype(np.float32)
        cos_g = (8.0 * cos * g[None, :]).astype(np.float16)
        sin_eff = (8.0 * np.concatenate([-sin[:, :32] * g[None, 32:],
                                         sin[:, 32:] * g[None, :32]],
                                        axis=-1)).astype(np.float16)
        outs += [cos_g, sin_eff]
    return outs  # cos_q, sin_q, cos_k, sin_k


def _make_in_maps(x, w_qkv, w_out, q_gamma, k_gamma):
    cos_q, sin_q, cos_k, sin_k = _host_tables(q_gamma, k_gamma)
    mask = (np.arange(128)[None, :] >= np.arange(128)[:, None]).astype(np.float16)

    in_maps = []
    for core in range(N_CORES):
        b, g = core // 4, core % 4
        xT = np.ascontiguousarray(x[b].T).astype(np.float16)
        wq = w_qkv[512 * g:512 * (g + 1)]                      # 8 q heads
        wk = w_qkv[2048 + 128 * g:2048 + 128 * (g + 1)]        # 2 k heads
        wv = w_qkv[2560 + 128 * g:2560 + 128 * (g + 1)]        # 2 v heads
        wqkvT = np.ascontiguousarray(
            np.concatenate([wq, wk, wv], axis=0).T).astype(np.float16)
        woutT = np.ascontiguousarray(
            w_out[:, 512 * g:512 * (g + 1)].T).astype(np.float16)
        in_maps.append({
            "xT": xT, "wqkvT": wqkvT, "woutT": woutT,
            "cos_q": cos_q, "sin_q": sin_q, "cos_k": cos_k, "sin_k": sin_k,
            "mask_i": mask,
        })
    return in_maps


def kernel(x, w_qkv, w_out, q_gamma, k_gamma):
    x = np.asarray(x)
    w_qkv = np.asarray(w_qkv)
    w_out = np.asarray(w_out)
    q_gamma = np.asarray(q_gamma)
    k_gamma = np.asarray(k_gamma)
    in_maps = _make_in_maps(x, w_qkv, w_out, q_gamma, k_gamma)
    results = _get_runner()(in_maps)
    parts = [results[c]["out"] for c in range(N_CORES)]
    out = np.empty((B, S, D), dtype=np.float32)
    for b in range(B):
        out[b] = parts[4 * b] + parts[4 * b + 1] + parts[4 * b + 2] + parts[4 * b + 3]
    return out


# revision 15
# speedup vs baseline: 1.2294x; 1.2294x over previous
"""Multi-head causal attention (GQA + QK-RMSNorm + RoPE) for Trainium2, 8 cores.

Sharding: 8 cores = 2 batches x 4 head-groups (tensor parallel over heads).
Each core handles one batch and 8 Q heads + 2 KV heads:
  - QKV projection for its head slice (fp16 matmuls, fp32 psum)
  - QK RMSNorm + RoPE (gamma folded into host-side cos/sin tables)
  - causal attention in k-major (transposed-scores) layout:
      sT[k, q] = kT.T @ qT ; p = exp(sT/8) ; ctx = pT.T @ [v | 1]
    (the ones column yields the softmax denominator for free)
  - output projection against its w_out column slice -> partial [S, D]
Host sums the 4 head-group partials per batch.

Layout/engine choices (v2):
  - all transposes (q/k after rope, ctx before out-proj) go through the DMA
    XBAR transpose (SBUF->SBUF, 16x128 tiles) instead of PE transpose +
    psum->SBUF copy; K heads are duplicated in the rope output so the
    transposed kT tile directly carries the [dims x2] layout the paired-head
    score matmuls need
  - attention J-blocks are interleaved into the QKV tile loop (attn_J(J)
    right after token tile 4J+3) so the scalar engine's exp overlaps the
    QKV matmul stream
  - score matmuls on diagonal blocks only stream the causally-live columns
  - cos/sin tables are fp16 so the rope multiplies hit DVE fast modes
"""
import sys
import numpy as np
from contextlib import ExitStack

if '/opt/trn_rl_repo' not in sys.path:
    sys.path.insert(0, '/opt/trn_rl_repo')

import concourse.bacc as bacc
import concourse.tile as tile
import concourse.mybir as mybir
from concourse.bass_utils import run_bass_kernel_spmd

dt = mybir.dt
AF = mybir.ActivationFunctionType
AX = mybir.AxisListType
ALU = mybir.AluOpType

HEAD_DIM = 64
NUM_Q_HEADS = 32
NUM_KV_HEADS = 8
ROPE_FREQ = 10000.0
EPS = 1e-6

B, S, D = 2, 2048, 2048
QH = 8            # q heads per core
KVH = 2           # kv heads per core
N_CORES = 8
ST = S // 128      # 16 token tiles of 128
NJ = S // 512      # 4 big q blocks of 512

F16 = dt.float16
F32 = dt.float32


def _build(repeat=1):
    nc = bacc.Bacc("TRN2", target_bir_lowering=False, debug=False,
                   num_devices=N_CORES)

    xT = nc.dram_tensor("xT", [D, S], F16, kind="ExternalInput").ap()
    wqkvT = nc.dram_tensor("wqkvT", [D, (QH + 2 * KVH) * HEAD_DIM], F16,
                           kind="ExternalInput").ap()          # cols: 512 q | 128 k | 128 v
    woutT = nc.dram_tensor("woutT", [QH * HEAD_DIM, D], F16,
                           kind="ExternalInput").ap()
    cos_q = nc.dram_tensor("cos_q", [S, HEAD_DIM], F16, kind="ExternalInput").ap()
    sin_q = nc.dram_tensor("sin_q", [S, HEAD_DIM], F16, kind="ExternalInput").ap()
    cos_k = nc.dram_tensor("cos_k", [S, HEAD_DIM], F16, kind="ExternalInput").ap()
    sin_k = nc.dram_tensor("sin_k", [S, HEAD_DIM], F16, kind="ExternalInput").ap()
    mask_i = nc.dram_tensor("mask_i", [128, 128], F16, kind="ExternalInput").ap()
    out = nc.dram_tensor("out", [S, D], F32, kind="ExternalOutput").ap()

    with tile.TileContext(nc) as tc, ExitStack() as ctx:
        perm = ctx.enter_context(tc.tile_pool(name="perm", bufs=1))

        # ---- persistent tiles ----
        w_ch = [perm.tile([128, 768], F16, tag=f"w{dc}", name=f"w{dc}") for dc in range(16)]
        wo_ch = [perm.tile([128, D], F16, tag=f"wo{p}", name=f"wo{p}") for p in range(4)]
        tabs = {}
        for nm in ("cq", "sq", "ck", "sk"):
            tabs[nm] = perm.tile([128, ST, HEAD_DIM], F16, tag=f"tab{nm}", name=f"tab{nm}")
        tab_srcs = {"cq": cos_q, "sq": sin_q, "ck": cos_k, "sk": sin_k}
        tmask = perm.tile([128, 128], F16, tag="mask")
        kmagic = perm.tile([128, 10], dt.int32, tag="kmagic")
        nc.vector.memset(kmagic[:], 0x5F3759DF)

        qTa = perm.tile([128, 4, S], F16, tag="qTa", name="qTa")
        kTa = perm.tile([128, KVH, S], F16, tag="kTa", name="kTa")
        vext = [perm.tile([128, KVH, HEAD_DIM + 1], F16, tag=f"v{i}", name=f"v{i}")
                for i in range(ST)]
        for i in range(ST):
            nc.vector.memset(vext[i][:, :, HEAD_DIM:HEAD_DIM + 1], 1.0)
        ctxTa = perm.tile([128, 4, S], F16, tag="ctxTa", name="ctxTa")

        rep_ctx = tc.For_i(0, repeat, 1) if repeat > 1 else None
        if rep_ctx is not None:
            ctx.enter_context(rep_ctx)

        # ---- pools; PSUM budget (8 banks): sT2 2x[128,1024] = 4,
        #      cbank 2x[128,260] = 2, shared po/p1-projection 2x[128,512] = 2
        psum = ctx.enter_context(tc.tile_pool(name="psum", bufs=2, space="PSUM"))
        p1sb = ctx.enter_context(tc.tile_pool(name="p1sb", bufs=4))
        p1st = ctx.enter_context(tc.tile_pool(name="p1st", bufs=2))
        ptpool = ctx.enter_context(tc.tile_pool(name="ptpool", bufs=6))
        cnpool = ctx.enter_context(tc.tile_pool(name="cnpool", bufs=3))
        rcpool = ctx.enter_context(tc.tile_pool(name="rcpool", bufs=8))
        osb = ctx.enter_context(tc.tile_pool(name="osb", bufs=6))

        def load_xt(i2):
            ta = p1sb.tile([128, 8, 256], F16, tag="xta", name="xta")
            tb = p1sb.tile([128, 8, 256], F16, tag="xtb", name="xtb")
            nc.sync.dma_start(
                ta[:], xT[0:1024, 256 * i2:256 * (i2 + 1)]
                .rearrange("(n p) s -> p n s", p=128))
            nc.sync.dma_start(
                tb[:], xT[1024:2048, 256 * i2:256 * (i2 + 1)]
                .rearrange("(n p) s -> p n s", p=128))
            return (ta, tb)

        for dc in range(4):
            nc.sync.dma_start(w_ch[dc][:], wqkvT[128 * dc:128 * (dc + 1), :])
        xts = {0: load_xt(0)}
        for dc in range(4, 16):
            nc.sync.dma_start(w_ch[dc][:], wqkvT[128 * dc:128 * (dc + 1), :])
        xts[1] = load_xt(1)
        for nm in ("cq", "sq", "ck", "sk"):
            nc.sync.dma_start(tabs[nm][:],
                              tab_srcs[nm].rearrange("(n p) d -> p n d", p=128))
        nc.sync.dma_start(tmask[:], mask_i[:])
        for p in range(4):
            nc.sync.dma_start(wo_ch[p][:], woutT[128 * p:128 * (p + 1), :])

        # ---------------- output projection drain queue ----------------
        po_queue = []

        def emit_po(i, dch):
            po = psum.tile([128, 512], F32, tag="tp", name="po")
            for p in range(4):
                nc.tensor.matmul(
                    po[:], ctxTa[:, p, 128 * i:128 * (i + 1)],
                    wo_ch[p][:, 512 * dch:512 * (dch + 1)],
                    start=(p == 0), stop=(p == 3))
            ob = osb.tile([128, 512], F32, tag="ob", name="ob")
            nc.vector.tensor_copy(ob[:], po[:])
            nc.sync.dma_start(
                out[128 * i:128 * (i + 1), 512 * dch:512 * (dch + 1)],
                ob[:])

        def pop_po(n=1):
            for _ in range(n):
                if po_queue:
                    emit_po(*po_queue.pop(0))

        # ---------------- phase-1 tile as a chunked generator -----------------
        def p1_tile_steps(i):
            """QKV + norm + rope for token tile i, yielding between chunks so
            the caller can interleave attention c-iterations."""
            xta, xtb = xts[i // 2]
            ii = i % 2
            if i % 2 == 0 and (i // 2) + 2 < ST // 2:
                xts[(i // 2) + 2] = load_xt((i // 2) + 2)

            def xs_(dc):
                t = xta if dc < 8 else xtb
                return t[:, dc % 8, 128 * ii:128 * (ii + 1)]
            # q projection: [128,512] accumulated in a shared "tp" psum slot
            pq = psum.tile([128, 512], F32, tag="tp", name="pq")
            for dcc in range(4):
                for dc in range(4 * dcc, 4 * dcc + 4):
                    nc.tensor.matmul(pq[:], xs_(dc),
                                     w_ch[dc][:, 0:512],
                                     start=(dc == 0), stop=(dc == 15))
                yield
            qsb = p1sb.tile([128, 640], F16, tag="qsb")
            nc.scalar.copy(qsb[:, 0:512], pq[:])
            sqt = p1sb.tile([128, 640], F16, tag="sqt")
            nc.vector.tensor_mul(sqt[:, 0:512], qsb[:, 0:512], qsb[:, 0:512])
            yield
            # kv projection: [128,256] in the next shared slot
            pkv = psum.tile([128, 256], F32, tag="tp", name="pkv")
            for dcc in range(2):
                for dc in range(8 * dcc, 8 * dcc + 8):
                    nc.tensor.matmul(pkv[:], xs_(dc),
                                     w_ch[dc][:, 512:768],
                                     start=(dc == 0), stop=(dc == 15))
                yield
            nc.scalar.copy(qsb[:, 512:640], pkv[:, 0:128])
            nc.vector.tensor_mul(sqt[:, 512:640], qsb[:, 512:640], qsb[:, 512:640])
            nc.scalar.copy(vext[i][:, :, 0:HEAD_DIM],
                           pkv[:, 128:256].rearrange("p (h d) -> p h d", h=2))
            yield
            # --- RMSNorm stats: reduce + Newton rsqrt (DVE)
            ssum = p1st.tile([128, 10], F32, tag="ssum")
            nc.vector.tensor_reduce(
                ssum[:], sqt[:].rearrange("p (h d) -> p h d", h=10),
                axis=AX.X, op=ALU.add)
            # rsqrt(ssum) = 1/(8*rms) -- the x8 in the rope tables completes
            # the normalization. Quake bit-shift seed + 2 Newton iterations
            # is <1e-5 relative for any positive input.
            hh_ = p1st.tile([128, 10], F32, tag="hh")
            nc.vector.tensor_scalar_mul(hh_[:], ssum[:], -0.5)
            sbi = p1st.tile([128, 10], dt.int32, tag="sbi")
            nc.vector.tensor_scalar(sbi[:], ssum[:].bitcast(dt.int32), 1, None,
                                    ALU.arith_shift_right)
            rstd = p1st.tile([128, 10], F32, tag="rstd")
            nc.vector.tensor_sub(rstd[:].bitcast(dt.int32), kmagic[:], sbi[:])
            ita = p1st.tile([128, 10], F32, tag="ita")
            itb = p1st.tile([128, 10], F32, tag="itb")
            for _ in range(2):
                nc.vector.tensor_mul(ita[:], rstd[:], rstd[:])
                nc.vector.tensor_mul(itb[:], ita[:], hh_[:])
                nc.vector.tensor_scalar_add(itb[:], itb[:], 1.5)
                nc.vector.tensor_mul(rstd[:], rstd[:], itb[:])
            yield
            # --- RoPE on unnormalized projections (tables carry x8)
            qru = p1sb.tile([128, 640], F16, tag="qru")
            t2 = p1sb.tile([128, 640], F16, tag="t2")
            qn_q4 = qsb[:, 0:512].rearrange("p (h two x) -> p h two x", h=8, two=2)
            qn_q3 = qsb[:, 0:512].rearrange("p (h d) -> p h d", h=8)
            cosq = tabs["cq"][:, i, :].unsqueeze(1).broadcast_to([128, 8, HEAD_DIM])
            sinq4 = tabs["sq"][:, i, :].unsqueeze(1) \
                .broadcast_to([128, 8, HEAD_DIM]) \
                .rearrange("p h (two x) -> p h two x", two=2)
            t2_q4 = t2[:, 0:512].rearrange("p (h two x) -> p h two x", h=8, two=2)
            nc.vector.tensor_mul(t2_q4[:, :, 0, :], qn_q4[:, :, 1, :], sinq4[:, :, 0, :])
            nc.vector.tensor_mul(t2_q4[:, :, 1, :], qn_q4[:, :, 0, :], sinq4[:, :, 1, :])
            qru_q3 = qru[:, 0:512].rearrange("p (h d) -> p h d", h=8)
            nc.vector.tensor_mul(qru_q3, qn_q3, cosq)
            nc.vector.tensor_add(qru[:, 0:512], qru[:, 0:512], t2[:, 0:512])
            yield
            qn_k4 = qsb[:, 512:640].rearrange("p (h two x) -> p h two x", h=2, two=2)
            qn_k3 = qsb[:, 512:640].rearrange("p (h d) -> p h d", h=2)
            cosk = tabs["ck"][:, i, :].unsqueeze(1).broadcast_to([128, 2, HEAD_DIM])
            sink4 = tabs["sk"][:, i, :].unsqueeze(1) \
                .broadcast_to([128, 2, HEAD_DIM]) \
                .rearrange("p h (two x) -> p h two x", two=2)
            t2k4 = t2[:, 512:640].rearrange("p (h two x) -> p h two x", h=2, two=2)
            qruk = qru[:, 512:640].rearrange("p (h d) -> p h d", h=2)
            nc.vector.tensor_mul(t2k4[:, :, 0, :], qn_k4[:, :, 1, :], sink4[:, :, 0, :])
            nc.vector.tensor_mul(t2k4[:, :, 1, :], qn_k4[:, :, 0, :], sink4[:, :, 1, :])
            nc.vector.tensor_mul(qruk, qn_k3, cosk)
            nc.vector.tensor_add(qru[:, 512:640], qru[:, 512:640], t2[:, 512:640])
            # --- join: qr = qru * rstd; k written into both dup slots
            qr = p1sb.tile([128, 768], F16, tag="qr")
            nc.vector.tensor_mul(
                qr[:, 0:512].rearrange("p (h d) -> p h d", h=8),
                qru[:, 0:512].rearrange("p (h d) -> p h d", h=8),
                rstd[:, 0:8].unsqueeze(2).broadcast_to([128, 8, HEAD_DIM]))
            qrk = qr[:, 512:768].rearrange("p (h dup d) -> p h dup d", h=2, dup=2)
            for d_ in (0, 1):
                nc.vector.tensor_mul(
                    qrk[:, :, d_, :],
                    qru[:, 512:640].rearrange("p (h d) -> p h d", h=2),
                    rstd[:, 8:10].unsqueeze(2).broadcast_to([128, 2, HEAD_DIM]))
            yield
            # --- transposes to dim-major via DMA XBAR (Act hwdge queue)
            nc.sync.dma_start(qTa[:, :, 128 * i:128 * (i + 1)],
                              qr[:, 0:512], transpose=True)
            nc.sync.dma_start(kTa[:, :, 128 * i:128 * (i + 1)],
                              qr[:, 512:768], transpose=True)
            yield

        def p1_group_steps(G):
            for i in range(4 * G, 4 * G + 4):
                yield from p1_tile_steps(i)

        # ---------------- attention J-block, p1 chunks injected per c ----------
        def attn_J(J, p1gen):
            n_c = 4 * (4 * J + 4)          # total c-iters this block
            n_pulled = [0]
            p1_chunks = []
            if p1gen is not None:
                p1_chunks.append(p1gen)

            def pull(frac):
                # keep the injected p1 stream proportional to attn progress
                if not p1_chunks:
                    return
                gen = p1_chunks[0]
                target = int(min(frac / 0.7, 1.0) * 29 * 4)
                while n_pulled[0] < target:
                    try:
                        next(gen)
                        n_pulled[0] += 1
                    except StopIteration:
                        p1_chunks.pop(0)
                        return

            ctxn = [cnpool.tile([128, 512], F16, tag=f"cn{jj}", name=f"cn{jj}")
                    for jj in range(4)]
            ci = [0]
            for hp in range(4):          # heads (2hp, 2hp+1), both use kv g
                g = hp // 2
                cbank = [psum.tile([128, 260], F32, tag="pkv", name=f"cb{w}")
                         for w in (0, 1)]

                def epi(jj):
                    w, loc = jj // 2, jj % 2
                    cb2 = cbank[w][:, 130 * loc:130 * (loc + 1)].rearrange(
                        "p (h e) -> p h e", h=2)
                    rc = rcpool.tile([128, 2], F32, tag="rc", name="rc")
                    nc.vector.reciprocal(rc[:], cb2[:, :, 64:65].squeeze(2))
                    nc.vector.tensor_mul(
                        ctxn[jj][:, 128 * hp:128 * (hp + 1)].rearrange(
                            "p (h d) -> p h d", h=2),
                        cb2[:, :, 0:64],
                        rc[:].unsqueeze(2).broadcast_to([128, 2, HEAD_DIM]))

                def ctx_mms(pt, jj0, c):
                    for jj in range(jj0, 4):
                        w, loc = jj // 2, jj % 2
                        for hh in (0, 1):
                            o = 130 * loc + 65 * hh
                            nc.tensor.matmul(
                                cbank[w][:, o:o + 65],
                                pt[:, 512 * hh + 128 * jj:512 * hh + 128 * (jj + 1)],
                                vext[c][:, g, :],
                                start=(c == 0 and loc == 0 and hh == 0),
                                stop=(c == 4 * J + jj and jj % 2 == 1 and hh == 1),
                                skip_group_check=True)

                pending = None
                for c in range(4 * J + 4):
                    jj0 = max(0, c - 4 * J)
                    sT2 = psum.tile([128, 1024], F32, tag="pq", name="sT2")
                    for hh in (0, 1):
                        nc.tensor.matmul(
                            sT2[:, 512 * hh + 128 * jj0:512 * (hh + 1)],
                            kTa[64 * hh:64 * hh + 64, g, 128 * c:128 * (c + 1)],
                            qTa[64 * hh:64 * hh + 64, hp,
                                512 * J + 128 * jj0:512 * (J + 1)],
                            start=True, stop=True)
                    if pending is not None:
                        ctx_mms(*pending)
                        pending = None
                        if c == 4 * J + 2:   # bank 0 (jj 0,1) is complete
                            epi(0)
                            epi(1)
                            if hp == 3:
                                for jj in (0, 1):
                                    nc.sync.dma_start(
                                        ctxTa[:, :, 512 * J + 128 * jj:
                                              512 * J + 128 * (jj + 1)],
                                        ctxn[jj][:, 0:512], transpose=True)
                    ci[0] += 1
                    if J == NJ - 1 and ci[0] % 4 != 0:
                        pop_po(1)
                    pull(ci[0] / n_c)
                    pt = ptpool.tile([128, 1024], F16, tag="pt", name="pt")
                    ptv = pt[:].rearrange("p (h x) -> p h x", h=2)[:, :, 128 * jj0:512]
                    sTv = sT2[:].rearrange("p (h x) -> p h x", h=2)[:, :, 128 * jj0:512]
                    nc.scalar.activation(ptv, sTv, AF.Exp, scale=0.125)
                    if c >= 4 * J:      # diagonal: triangular mask, both heads
                        dv = pt[:].rearrange("p (h x) -> p h x", h=2)[
                            :, :, 128 * jj0:128 * (jj0 + 1)]
                        nc.vector.tensor_mul(
                            dv, dv,
                            tmask[:].unsqueeze(1).broadcast_to([128, 2, 128]))
                    pending = (pt, jj0, c)
                ctx_mms(*pending)
                pending = None
                for jj in (2, 3):
                    epi(jj)
            # ctx tiles to dim-major via DMA XBAR (jj 0,1 already issued)
            for jj in (2, 3):
                nc.sync.dma_start(
                    ctxTa[:, :, 512 * J + 128 * jj:512 * J + 128 * (jj + 1)],
                    ctxn[jj][:, 0:512], transpose=True)
            # drain any p1 remainder, then queue this block's out-projection
            while p1_chunks:
                pull(2.0)
            for i in range(4 * J, 4 * J + 4):
                for dch in range(4):
                    po_queue.append((i, dch))

        # ---------------- main schedule -----------------
        for _ in p1_group_steps(0):
            pass
        for J in range(NJ):
            attn_J(J, p1_group_steps(J + 1) if J + 1 < NJ else None)
        while po_queue:
            emit_po(*po_queue.pop(0))

    nc.compile()
    return nc


_NC = {}


def _get_nc(repeat=1):
    if repeat not in _NC:
        _NC[repeat] = _build(repeat)
    return _NC[repeat]


_RUNNER = {}


def _get_runner(repeat=1):
    """Build (once) a jitted 8-core sharded callable around the bass program.

    Slim replica of bass2jax.run_bass_via_pjrt's multi-core path, kept
    reusable so repeated invocations skip retracing/recompilation.
    """
    if repeat in _RUNNER:
        return _RUNNER[repeat]
    import jax
    from jax.sharding import Mesh, PartitionSpec
    from jax.experimental.shard_map import shard_map
    from concourse import bass2jax
    from concourse import mybir as _mybir

    nc = _get_nc(repeat)
    bass2jax.install_neuronx_cc_hook()

    partition_name = nc.partition_id_tensor.name if nc.partition_id_tensor else None
    in_names, out_names, out_avals, zero_outs = [], [], [], []
    for alloc in nc.m.functions[0].allocations:
        if not isinstance(alloc, _mybir.MemoryLocationSet):
            continue
        name = alloc.memorylocations[0].name
        if alloc.kind == "ExternalInput":
            if name != partition_name:
                in_names.append(name)
        elif alloc.kind == "ExternalOutput":
            shape = tuple(alloc.tensor_shape)
            np_dt = _mybir.dt.np(alloc.dtype)
            out_names.append(name)
            out_avals.append(jax.core.ShapedArray(shape, np_dt))
            zero_outs.append(np.zeros(shape, np_dt))
    n_params = len(in_names)
    all_in_names = list(in_names) + list(out_names)
    if partition_name is not None:
        all_in_names.append(partition_name)

    def _body(*args):
        operands = list(args)
        if partition_name is not None:
            operands.append(bass2jax.partition_id_tensor())
        outs = bass2jax._bass_exec_p.bind(
            *operands,
            out_avals=tuple(out_avals),
            in_names=tuple(all_in_names),
            out_names=tuple(out_names),
            lowering_input_output_aliases=(),
            sim_require_finite=True,
            sim_require_nnan=True,
            nc=nc,
        )
        return tuple(outs)

    devices = jax.devices()[:N_CORES]
    mesh = Mesh(np.asarray(devices), ("core",))
    in_specs = (PartitionSpec("core"),) * (n_params + len(out_names))
    out_specs = (PartitionSpec("core"),) * len(out_names)
    sharded = jax.jit(shard_map(_body, mesh=mesh, in_specs=in_specs,
                                out_specs=out_specs, check_rep=False),
                      keep_unused=True)

    concat_zeros = [np.zeros((N_CORES * z.shape[0], *z.shape[1:]), z.dtype)
                    for z in zero_outs]

    _dev_cache = {}

    def run(in_maps, iters=1, time_list=None, fetch=True):
        import time as _time
        from jax.sharding import NamedSharding
        shard = NamedSharding(mesh, PartitionSpec("core"))
        key = id(in_maps)
        if key not in _dev_cache:
            per_core = [[np.asarray(m[nm]) for nm in in_names] for m in in_maps]
            concat_in = [np.concatenate([per_core[c][i] for c in range(N_CORES)],
                                        axis=0) for i in range(n_params)]
            dev_in = [jax.device_put(a, shard) for a in concat_in]
            dev_zero = [jax.device_put(z, shard) for z in concat_zeros]
            jax.block_until_ready(dev_in)
            _dev_cache.clear()
            _dev_cache[key] = (dev_in, dev_zero)
        dev_in, dev_zero = _dev_cache[key]
        out_arrs = None
        if iters <= 1:
            out_arrs = sharded(*dev_in, *dev_zero)
            jax.block_until_ready(out_arrs)
        else:
            # async batch: submit all, block once; caller computes slope
            sharded(*dev_in, *dev_zero)  # warm
            t0 = _time.perf_counter()
            for _ in range(iters):
                out_arrs = sharded(*dev_in, *dev_zero)
            jax.block_until_ready(out_arrs)
            if time_list is not None:
                time_list.append(_time.perf_counter() - t0)
        if not fetch:
            del out_arrs
            return None
        return [
            {nm: np.asarray(out_arrs[i]).reshape(N_CORES, *out_avals[i].shape)[c]
             for i, nm in enumerate(out_names)}
            for c in range(N_CORES)
        ]

    _RUNNER[repeat] = run
    return run


def _host_tables(q_gamma, k_gamma):
    pos = np.arange(S, dtype=np.float32)
    inv = 1.0 / (ROPE_FREQ ** (np.arange(0, HEAD_DIM, 2, dtype=np.float32)
                               / HEAD_DIM))
    fr = pos[:, None] * inv[None, :]
    emb = np.concatenate([fr, fr], axis=-1)
    cos = np.cos(emb).astype(np.float32)
    sin = np.sin(emb).astype(np.float32)
    outs = []
    for gamma in (q_gamma, k_gamma):
        g = gamma.astype(np.float32)
        cos_g = (8.0 * cos * g[None, :]).astype(np.float16)
        sin_eff = (8.0 * np.concatenate([-sin[:, :32] * g[None, 32:],
                                         sin[:, 32:] * g[None, :32]],
                                        axis=-1)).astype(np.float16)
        outs += [cos_g, sin_eff]
    return outs  # cos_q, sin_q, cos_k, sin_k


def _make_in_maps(x, w_qkv, w_out, q_gamma, k_gamma):
    cos_q, sin_q, cos_k, sin_k = _host_tables(q_gamma, k_gamma)
    mask = (np.arange(128)[None, :] >= np.arange(128)[:, None]).astype(np.float16)

    in_maps = []
    for core in range(N_CORES):
        b, g = core // 4, core % 4
        xT = np.ascontiguousarray(x[b].T).astype(np.float16)
        wq = w_qkv[512 * g:512 * (g + 1)]                      # 8 q heads
        wk = w_qkv[2048 + 128 * g:2048 + 128 * (g + 1)]        # 2 k heads
        wv = w_qkv[2560 + 128 * g:2560 + 128 * (g + 1)]        # 2 v heads
        wqkvT = np.ascontiguousarray(
            np.concatenate([wq, wk, wv], axis=0).T).astype(np.float16)
        woutT = np.ascontiguousarray(
            w_out[:, 512 * g:512 * (g + 1)].T).astype(np.float16)
        in_maps.append({
            "xT": xT, "wqkvT": wqkvT, "woutT": woutT,
            "cos_q": cos_q, "sin_q": sin_q, "cos_k": cos_k, "sin_k": sin_k,
            "mask_i": mask,
        })
    return in_maps


def kernel(x, w_qkv, w_out, q_gamma, k_gamma):
    x = np.asarray(x)
    w_qkv = np.asarray(w_qkv)
    w_out = np.asarray(w_out)
    q_gamma = np.asarray(q_gamma)
    k_gamma = np.asarray(k_gamma)
    in_maps = _make_in_maps(x, w_qkv, w_out, q_gamma, k_gamma)
    results = _get_runner()(in_maps)
    parts = [results[c]["out"] for c in range(N_CORES)]
    out = np.empty((B, S, D), dtype=np.float32)
    for b in range(B):
        out[b] = parts[4 * b] + parts[4 * b + 1] + parts[4 * b + 2] + parts[4 * b + 3]
    return out
